# revision 1
# baseline (speedup 1.0000x reference)
"""Multi-head attention (relu + valid-key-count normalization) on 8 TRN2 cores.

Strategy: data-parallel over batch (B=16 -> 2 per core), no collectives.
All matmul operands are float16 (full PE rate; 11-bit mantissa keeps the
end-to-end rel err ~1e-3 against the 2e-2 gate).

Key transformations vs the v1 baseline (367.7us):
- The reference normalization collapses to
      A_final[q,k] = relu(A[q,k]) * mask[q,k] * scale / max(m[q],1),
  and the whole multiplicative factor maskq[k,q] = mask[q,k]*scale/max(m,1)
  is precomputed ON HOST as an f16 tensor (transposed to match the
  k-on-partitions logits layout). This removes the on-device mask casts,
  mask transposes (8,192 PE cyc/batch), the m[q] accumulation, the qs
  broadcast chain, and the per-head Q'-scale multiply. relu+mask+norm is
  a single DVE scalar_tensor_tensor per logits tile, straight from PSUM.
- V^T is computed directly on the PE as x-stationary matmuls
  (V^T[k,c] = sum_u x[u,k] wv[u,c]) instead of V followed by PE
  transposes: same matmul cycles, minus 8,192 transpose cyc/batch.
- Weights are loaded once per core (f16, host-packed per head so every
  DMA is a full-contiguous descriptor) and stay resident in SBUF for
  both batches; x/mask DMAs are f16 (half the bytes of v1).
- All loads ride one DMA queue (SP) in explicit priority order (x half 0
  + head-0 weights first); out stores ride SP too so the ACT sequencer
  only does PSUM->SBUF copies. A chain of warm-up transposes fed by a
  DVE memset keeps the PE busy from ~1us until the first real matmul —
  otherwise the p-state ramp restarts after the prologue idle and the
  first ~6us of real matmuls run at 0.65-1.2GHz instead of 2.4.
- Batch b+1's head-0 QKV is emitted before batch b's last-head
  attention as scheduler filler for the DVE STT backlog; the final
  output tile is computed as 4 column-group accumulations so the drain
  tail ends on one small copy + store.

Timeline: 336.7us = ~4.7us DMA-bound prologue (bridged by warm-up;
head-0 weights + the first x chunks are split so the Q chain starts on
partial data) + 326.9us PE busy (786,432 cycles @2.4GHz = the f16
matmul floor) + ~0.9us scheduling gaps + 3.9us store/drain tail.
fp8 DoubleRow (0.5 cyc/row) was measured end-to-end in numpy at
3.2-6.1e-2 rel err for every matmul site — all above the 2e-2 gate —
so f16 at 1.0 cyc/row is the fastest admissible dtype. Measured rel
err vs the jax reference: ~7e-4 (gate 2e-2).
"""
import sys

sys.path.insert(0, "/opt/trn_rl_repo")

import numpy as np

import concourse.bacc as bacc
import concourse.mybir as mybir
import concourse.tile as tile
from concourse.bass_utils import run_bass_kernel_spmd

B, U, S, H, C = 16, 1024, 1024, 8, 128
NCORES = 8
BPC = B // NCORES  # batches per core
SCALE = float(1.0 / np.sqrt(np.float32(C)))
P = 128  # partitions
UC = U // P  # u chunks
QT = S // P  # q tiles
KT = S // P  # k tiles
NH = 512  # matmul free dim (one PSUM bank of f32)
NWARM = 34  # p-state warm-up transposes: bridge PE busy ~1us -> first matmul

F32 = mybir.dt.float32
F16 = mybir.dt.float16


def build():
    nc = bacc.Bacc()
    x_d = nc.dram_tensor("x16", [BPC, UC, P, S], F16, kind="ExternalInput")
    mq_d = nc.dram_tensor("maskq", [BPC, KT, P, S], F16, kind="ExternalInput")
    wq_d = nc.dram_tensor("wq", [H, P, UC, C], F16, kind="ExternalInput")
    wk_d = nc.dram_tensor("wk", [H, P, UC, C], F16, kind="ExternalInput")
    wv_d = nc.dram_tensor("wv", [H, P, UC, C], F16, kind="ExternalInput")
    wo_d = nc.dram_tensor("wo", [P, UC, U], F16, kind="ExternalInput")
    out_d = nc.dram_tensor("out", [BPC, U, S], F32, kind="ExternalOutput")

    xv = x_d[:].rearrange("b u p s -> b p u s")
    mqv = mq_d[:].rearrange("b k p s -> b p k s")

    with tile.TileContext(nc) as tc:
        with (
            tc.tile_pool(name="sb", bufs=1) as sb,
            tc.tile_pool(name="ps", bufs=1, space="PSUM") as ps,
        ):
            # warm-up: a DVE memset (no DMA dependency) feeds a chain of PE
            # transposes that keep the PE continuously busy from ~1us until
            # the first real matmul — otherwise the p-state ramp restarts
            # after the prologue idle and the first ~6us of real matmuls run
            # at 0.65-1.2GHz instead of 2.4GHz.
            wsrc = sb.tile([P, P], F16, tag="wsrc")
            nc.vector.memset(wsrc[:], 0.0)
            warm = ps.tile([P, P], F16, tag="a", bufs=2, name="warm")
            for _ in range(NWARM):
                nc.tensor.transpose(warm[:], wsrc[:], wsrc[:])

            # resident weights; head 0 first so QKV can start ASAP
            wq_sb = [
                sb.tile([P, UC, C], F16, tag=f"wq{h}", name=f"wq_sb{h}")
                for h in range(H)
            ]
            wk_sb = [
                sb.tile([P, UC, C], F16, tag=f"wk{h}", name=f"wk_sb{h}")
                for h in range(H)
            ]
            wv_sb = [
                sb.tile([P, UC, C], F16, tag=f"wv{h}", name=f"wv_sb{h}")
                for h in range(H)
            ]
            x_sb = [
                sb.tile([P, UC, S], F16, tag="x", bufs=2, name=f"x{b}")
                for b in range(BPC)
            ]
            mq_sb = [
                sb.tile([P, KT, S], F16, tag="mq", bufs=2, name=f"mq{b}")
                for b in range(BPC)
            ]
            # ALL loads on the SP queue in strict priority order — the DMA
            # engines device is serialized, so transfer order IS this order.
            # batch-0 x in column halves: the first QKV matmuls need only
            # half 0 of every uc chunk.
            nc.sync.dma_start(wq_sb[0][:], wq_d[0])
            nc.sync.dma_start(x_sb[0][:, 0:3, 0:NH], xv[0, :, 0:3, 0:NH])
            nc.sync.dma_start(x_sb[0][:, 3:6, 0:NH], xv[0, :, 3:6, 0:NH])
            nc.sync.dma_start(x_sb[0][:, 6:UC, 0:NH], xv[0, :, 6:UC, 0:NH])
            nc.sync.dma_start(wk_sb[0][:], wk_d[0])
            nc.sync.dma_start(wv_sb[0][:], wv_d[0])
            nc.sync.dma_start(x_sb[0][:, :, NH:S], xv[0, :, :, NH:S])
            for kc in range(4):
                nc.sync.dma_start(mq_sb[0][:, kc, :], mqv[0, :, kc, :])
            nc.sync.dma_start(wq_sb[1][:], wq_d[1])
            nc.sync.dma_start(wk_sb[1][:], wk_d[1])
            nc.sync.dma_start(wv_sb[1][:], wv_d[1])
            for kc in range(4, KT):
                nc.sync.dma_start(mq_sb[0][:, kc, :], mqv[0, :, kc, :])
            for h in range(2, H):
                nc.sync.dma_start(wq_sb[h][:], wq_d[h])
                nc.sync.dma_start(wk_sb[h][:], wk_d[h])
                nc.sync.dma_start(wv_sb[h][:], wv_d[h])
            wo_sb = sb.tile([P, UC, U], F16, tag="wo")
            nc.sync.dma_start(wo_sb[:], wo_d[:])
            if BPC > 1:
                nc.sync.dma_start(x_sb[1][:], xv[1])
                nc.sync.dma_start(mq_sb[1][:], mqv[1])

            def emit_qkv(b, h):
                qp = sb.tile([P, S], F16, tag="qp", bufs=2, name=f"qp{b}_{h}")
                ks = sb.tile([P, S], F16, tag="ks", bufs=2, name=f"ks{b}_{h}")
                vt = sb.tile([P, KT, C], F16, tag="vt", bufs=2, name=f"vt{b}_{h}")
                for half in range(2):
                    sl = slice(half * NH, (half + 1) * NH)
                    acc = ps.tile([P, NH], F32, tag="qk", bufs=4, name=f"accq{b}_{h}")
                    for uc in range(UC):
                        nc.tensor.matmul(
                            acc[:],
                            wq_sb[h][:, uc, :],
                            x_sb[b][:, uc, sl],
                            start=(uc == 0),
                            stop=(uc == UC - 1),
                        )
                    nc.scalar.copy(qp[:, sl], acc[:])
                    acc = ps.tile([P, NH], F32, tag="qk", bufs=4, name=f"acck{b}_{h}")
                    for uc in range(UC):
                        nc.tensor.matmul(
                            acc[:],
                            wk_sb[h][:, uc, :],
                            x_sb[b][:, uc, sl],
                            start=(uc == 0),
                            stop=(uc == UC - 1),
                        )
                    nc.scalar.copy(ks[:, sl], acc[:])
                    # V^T directly: stationary = x block, moving = wv
                    vtp = ps.tile([P, NH], F32, tag="qk", bufs=4, name=f"vtp{b}_{h}")
                    for j in range(4):
                        kc = half * 4 + j
                        for uc in range(UC):
                            nc.tensor.matmul(
                                vtp[:, j * C : (j + 1) * C],
                                x_sb[b][:, uc, kc * P : (kc + 1) * P],
                                wv_sb[h][:, uc, :],
                                start=(uc == 0),
                                stop=(uc == UC - 1),
                            )
                    nc.scalar.copy(
                        vt[:, half * 4 : (half + 1) * 4, :],
                        vtp[:].rearrange("p (j c) -> p j c", c=C),
                    )
                return qp, ks, vt

            def emit_oproj(ob, occ, ot, half):
                od = out_d[
                    ob, ot * P : (ot + 1) * P, half * NH : (half + 1) * NH
                ]
                o_ps = ps.tile(
                    [P, NH], F32, tag="qk", bufs=4, name=f"odf{ob}_{ot}_{half}"
                )
                for uc in range(UC):
                    nc.tensor.matmul(
                        o_ps[:],
                        wo_sb[:, uc, ot * P : (ot + 1) * P],
                        occ[:, uc, half * NH : (half + 1) * NH],
                        start=(uc == 0),
                        stop=(uc == UC - 1),
                    )
                o_sb = sb.tile(
                    [P, NH], F32, tag="o_sb", bufs=3, name=f"osdf{ob}_{ot}_{half}"
                )
                nc.scalar.copy(o_sb[:], o_ps[:])
                nc.sync.dma_start(od, o_sb[:])

            qkv_pre = None
            deferred = []
            for b in range(BPC):
                cc = sb.tile([P, H, S], F16, tag="cc", bufs=2, name=f"cc{b}")
                for h in range(H):
                    if qkv_pre is not None and qkv_pre[0] == (b, h):
                        qp, ks, vt = qkv_pre[1]
                        qkv_pre = None
                    else:
                        qp, ks, vt = emit_qkv(b, h)
                    if b + 1 < BPC and h == H - 1:
                        # pre-emit next batch's head-0 QKV: independent work
                        # the scheduler can use to fill the last head's
                        # attention-tail stalls (AV waiting on the DVE STT
                        # backlog) at the batch boundary
                        qkv_pre = ((b + 1, 0), emit_qkv(b + 1, 0))

                    # logits (transposed) + fused relu*maskq + AV accumulation
                    ch0 = ps.tile([P, NH], F32, tag="ch", bufs=2)
                    ch1 = ps.tile([P, NH], F32, tag="ch", bufs=2)
                    for kc in range(KT):
                        for half, ch in ((0, ch0), (1, ch1)):
                            a_ps = ps.tile([P, NH], F32, tag="a", bufs=2)
                            nc.tensor.matmul(
                                a_ps[:],
                                ks[:, kc * P : (kc + 1) * P],
                                qp[:, half * NH : (half + 1) * NH],
                                start=True,
                                stop=True,
                            )
                            atf = sb.tile([P, NH], F16, tag="atf", bufs=4)
                            nc.vector.scalar_tensor_tensor(
                                atf[:],
                                a_ps[:],
                                0.0,
                                mq_sb[b][:, kc, half * NH : (half + 1) * NH],
                                op0=mybir.AluOpType.max,
                                op1=mybir.AluOpType.mult,
                            )
                            nc.tensor.matmul(
                                ch[:],
                                vt[:, kc, :],
                                atf[:],
                                start=(kc == 0),
                                stop=(kc == KT - 1),
                            )
                    nc.scalar.copy(cc[:, h, 0:NH], ch0[:])
                    if h == H - 1:
                        # last head: half-1 copy on DVE (free after its final
                        # STT) so both copies run in parallel and the
                        # out-proj's uc=7 matmuls aren't serialized behind ACT
                        nc.vector.tensor_copy(cc[:, h, NH:S], ch1[:])
                    else:
                        nc.scalar.copy(cc[:, h, NH:S], ch1[:])

                # ---- output projection (weights already resident) ----
                # the first 2 tiles of every non-final batch are deferred to
                # just before the final batch's out-proj: they are the only
                # independent work available to fill the last head's
                # attention-tail stalls (the DVE STT backlog) there
                defer = (
                    {(0, 0), (0, 1)} if BPC > 1 and b < BPC - 1 else set()
                )
                if b == BPC - 1:
                    for db, dcc, dot, dhalf in deferred:
                        emit_oproj(db, dcc, dot, dhalf)
                for ot in range(UC):
                    for half in range(2):
                        if (ot, half) in defer:
                            deferred.append((b, cc, ot, half))
                            continue
                        od = out_d[
                            b,
                            ot * P : (ot + 1) * P,
                            half * NH : (half + 1) * NH,
                        ]
                        if b == BPC - 1 and ot == UC - 1 and half == 1:
                            # final tile: 4 column-group accumulations in
                            # separate PSUM tiles with interleaved copies, so
                            # after the last matmul only one 128-col copy and
                            # the single DMA remain
                            o_sb = sb.tile([P, NH], F32, tag="o_sb", bufs=3)
                            for j in range(4):
                                jsl = slice(j * P, (j + 1) * P)
                                op_j = ps.tile(
                                    [P, P], F32, tag="qk", bufs=4, name=f"opfin{j}"
                                )
                                for uc in range(UC):
                                    nc.tensor.matmul(
                                        op_j[:],
                                        wo_sb[:, uc, ot * P : (ot + 1) * P],
                                        cc[:, uc, half * NH + j * P : half * NH + (j + 1) * P],
                                        start=(uc == 0),
                                        stop=(uc == UC - 1),
                                    )
                                nc.scalar.copy(o_sb[:, jsl], op_j[:])
                                if j == 2:
                                    nc.sync.dma_start(
                                        od[:, 0 : 3 * P], o_sb[:, 0 : 3 * P]
                                    )
                            # last chunk alone on SP: its DGE_DMA_DELAY is
                            # 650ns vs ACT's 784, and the transfer is 128 cols
                            nc.sync.dma_start(od[:, 3 * P : NH], o_sb[:, 3 * P : NH])
                        else:
                            o_ps = ps.tile([P, NH], F32, tag="qk", bufs=4)
                            for uc in range(UC):
                                nc.tensor.matmul(
                                    o_ps[:],
                                    wo_sb[:, uc, ot * P : (ot + 1) * P],
                                    cc[:, uc, half * NH : (half + 1) * NH],
                                    start=(uc == 0),
                                    stop=(uc == UC - 1),
                                )
                            o_sb = sb.tile([P, NH], F32, tag="o_sb", bufs=3)
                            nc.scalar.copy(o_sb[:], o_ps[:])
                            nc.sync.dma_start(od, o_sb[:])

    nc.compile()
    return nc


_NC_CACHE = None


def _get_nc():
    global _NC_CACHE
    if _NC_CACHE is None:
        _NC_CACHE = build()
    return _NC_CACHE


def kernel(x, mask, w_qkv, w_out):
    nc = _get_nc()
    x = np.asarray(x, dtype=np.float32)
    mask_b = np.asarray(mask).astype(bool)
    w_qkv = np.asarray(w_qkv, dtype=np.float32)
    w_out = np.asarray(w_out, dtype=np.float32)

    # maskq[b,k,q] = mask[b,q,k] * scale / max(valid_count[b,q], 1)
    m = mask_b.sum(axis=2).astype(np.float32)  # [B, S]
    qs = SCALE / np.maximum(m, 1.0)
    maskq = mask_b.astype(np.float32) * qs[:, :, None]  # [B, q, k]
    mq = (
        np.ascontiguousarray(maskq.transpose(0, 2, 1))
        .astype(np.float16)
        .reshape(B, KT, P, S)
    )
    x16 = x.astype(np.float16).reshape(B, UC, P, S)

    wqkvT = np.ascontiguousarray(w_qkv.T).astype(np.float16)  # [U, 3U]
    packs = []
    for i in range(3):
        w_i = wqkvT[:, i * U : (i + 1) * U]  # [U, U]
        packs.append(
            np.ascontiguousarray(
                w_i.reshape(UC, P, H, C).transpose(2, 1, 0, 3)
            )  # [H, P, UC, C]
        )
    wq, wk, wv = packs
    wo = np.ascontiguousarray(
        w_out.T.astype(np.float16).reshape(UC, P, U).transpose(1, 0, 2)
    )  # [P, UC, U]

    in_maps = []
    for c in range(NCORES):
        in_maps.append(
            {
                "x16": np.ascontiguousarray(x16[c * BPC : (c + 1) * BPC]),
                "maskq": np.ascontiguousarray(mq[c * BPC : (c + 1) * BPC]),
                "wq": wq,
                "wk": wk,
                "wv": wv,
                "wo": wo,
            }
        )
    res = run_bass_kernel_spmd(nc, in_maps, list(range(NCORES)))
    out = np.concatenate([res.results[c]["out"] for c in range(NCORES)], axis=0)
    return out



# revision 3
# speedup vs baseline: 1.1552x; 1.1552x over previous
"""Multi-head attention (relu + valid-key-count normalization) on 8 TRN2 cores.

Strategy: data-parallel over batch (B=16 -> 2 per core), no collectives.

v3: fp8 DoubleRow 3-term residual matmuls for QKV + output projection.
The TRN2 PE in fp8e4 DoubleRow mode contracts 256 rows per instruction at
0.5 cyc per output column = 4x the f16 MAC rate. A plain fp8 cast is too
lossy (3-6e-2 end-to-end, gate 2e-2), but the 3-term residual scheme
    W ~ Wh + Wl,  X ~ Xh + Xl   (hi = e4m3(t), lo = e4m3(t - hi))
    W^T X ~ Wh^T Xh + (Wl^T Xh + Wh^T Xl)     [drop Wl^T Xl, ~2^-9]
costs 1.5 DoubleRow instructions per 256 contraction rows = 0.75x the f16
cycles, at ~2.2e-3 end-to-end rel err (measured in a bit-exact numpy sim
of this pipeline). Packing needs NO duplication:
  - main terms pair hi-blocks across the contraction dim:
        stat [Wh_j; Wh_j+1], mov [Xh_j; Xh_j+1]
  - both correction terms ride ONE DoubleRow via the cross pairing:
        stat [Wl_j; Wh_j],  mov [Xh_j; Xl_j]
All tensors quantize on a power-of-2 scale chosen so values AND residuals
sit in e4m3 normal range (subnormals at scale 1 destroyed the residuals:
2.8e-2 measured): x*32, w_qkv*1024, w_out*1024, cc*512. Descales fold
into the existing PSUM->SBUF copies (ACT mul) so there is zero extra
elementwise work; cc hi/lo is produced by the existing ACT copy (now a
mul to fp8) plus one DVE STT for the residual.

Logits (contraction = head dim 128 < 256) gains nothing from DoubleRow,
and fp8 attention weights would push DVE/ACT past the PE time, so
logits + AV stay f16: Q/K/V^T emerge from the fp8 QKV PSUM as f16 via
the descaling ACT mul.

Per-core PE floor: QKV 0.75*3 + logits 1 + AV 1 + oproj 0.75 = 5.0 units
of 65536 cyc/batch, 2 batches = 655,360 cyc @2.4GHz = 273.1us (f16 floor
was 327.7us; baseline measured 336.4us).

Carried over from v2: warm-up transpose chain bridging the DMA prologue
p-state ramp, single-queue (SP) DMA priority order with head-0 weights +
first x chunks split fine, next-batch head-0 QKV pre-emitted as
batch-boundary scheduler filler, first-2 out-proj tiles of batch 0
deferred to the final batch, final out tile drained as 4 column groups.
Measured rel err vs the jax reference: ~2e-3 (gate 2e-2).
"""
import sys

sys.path.insert(0, "/opt/trn_rl_repo")

import numpy as np
import ml_dtypes

import concourse.bacc as bacc
import concourse.mybir as mybir
import concourse.tile as tile
from concourse.bass_utils import run_bass_kernel_spmd

B, U, S, H, C = 16, 1024, 1024, 8, 128
NCORES = 8
BPC = B // NCORES  # batches per core
SCALE = float(1.0 / np.sqrt(np.float32(C)))
P = 128  # partitions
UC = U // P  # u chunks
QT = S // P  # q tiles
KT = S // P  # k tiles
NH = 512  # matmul free dim (one PSUM bank of f32)
NWARM = 34  # p-state warm-up transposes: bridge PE busy ~1us -> first matmul

SX = 32.0  # x fp8 scale (power of 2; absmax*SX must stay <= 240)
SW = 1024.0  # w_qkv fp8 scale
SWO = 1024.0  # w_out fp8 scale
SCC = 512.0  # on-device attention-output (cc) fp8 scale
INV_QK = float(1.0 / (SW * SX))  # QKV PSUM -> f16 descale
INV_O = float(1.0 / (SWO * SCC))  # out-proj PSUM -> f32 descale

F32 = mybir.dt.float32
F16 = mybir.dt.float16
F8 = mybir.dt.float8e4
E4M3 = ml_dtypes.float8_e4m3
DR = mybir.MatmulPerfMode.DoubleRow


def build():
    nc = bacc.Bacc()
    # Host-packed to SBUF layout (partition dim right after batch): every
    # load is one fully-contiguous descriptor per partition.
    # x8 dim2: {0: hi, 1: lo}; weights dim after P: {0: lo, 1: hi}.
    x_d = nc.dram_tensor("x8", [BPC, P, 2, UC, S], F8, kind="ExternalInput")
    mq_d = nc.dram_tensor("maskq", [BPC, P, KT, S], F16, kind="ExternalInput")
    wq_d = nc.dram_tensor("wq", [H, P, 2, UC, C], F8, kind="ExternalInput")
    wk_d = nc.dram_tensor("wk", [H, P, 2, UC, C], F8, kind="ExternalInput")
    wv_d = nc.dram_tensor("wv", [H, P, 2, UC, C], F8, kind="ExternalInput")
    wo_d = nc.dram_tensor("wo", [P, 2, UC, U], F8, kind="ExternalInput")
    out_d = nc.dram_tensor("out", [BPC, U, S], F32, kind="ExternalOutput")

    with tile.TileContext(nc) as tc:
        with (
            tc.tile_pool(name="sb", bufs=1) as sb,
            tc.tile_pool(name="ps", bufs=1, space="PSUM") as ps,
        ):
            # warm-up: a DVE memset (no DMA dependency) feeds a chain of PE
            # transposes that keep the PE continuously busy from ~1us until
            # the first real matmul — otherwise the p-state ramp restarts
            # after the prologue idle and the first ~6us of real matmuls run
            # at 0.65-1.2GHz instead of 2.4GHz.
            wsrc = sb.tile([P, P], F16, tag="wsrc")
            nc.vector.memset(wsrc[:], 0.0)
            warm = ps.tile([P, P], F16, tag="a", bufs=2, name="warm")
            for _ in range(NWARM):
                nc.tensor.transpose(warm[:], wsrc[:], wsrc[:])

            # resident weights; head 0 first so QKV can start ASAP
            wq_sb = [
                sb.tile([P, 2, UC, C], F8, tag=f"wq{h}", name=f"wq_sb{h}")
                for h in range(H)
            ]
            wk_sb = [
                sb.tile([P, 2, UC, C], F8, tag=f"wk{h}", name=f"wk_sb{h}")
                for h in range(H)
            ]
            wv_sb = [
                sb.tile([P, 2, UC, C], F8, tag=f"wv{h}", name=f"wv_sb{h}")
                for h in range(H)
            ]
            x_sb = [
                sb.tile([P, 2, UC, S], F8, tag="x", bufs=2, name=f"x{b}")
                for b in range(BPC)
            ]
            mq_sb = [
                sb.tile([P, KT, S], F16, tag="mq", bufs=2, name=f"mq{b}")
                for b in range(BPC)
            ]
            # ALL loads on the SP queue in strict priority order — the DMA
            # engines device is serialized, so transfer order IS this order.
            # batch-0 x in column halves: the first QKV matmuls need only
            # half 0 of every uc chunk (hi AND lo: the correction DRs
            # interleave with the main DRs in the same accumulation group).
            nc.sync.dma_start(wq_sb[0][:], wq_d[0])
            for t in range(2):
                nc.sync.dma_start(
                    x_sb[0][:, t, 0:3, 0:NH], x_d[0, :, t, 0:3, 0:NH]
                )
            for t in range(2):
                nc.sync.dma_start(
                    x_sb[0][:, t, 3:6, 0:NH], x_d[0, :, t, 3:6, 0:NH]
                )
            for t in range(2):
                nc.sync.dma_start(
                    x_sb[0][:, t, 6:UC, 0:NH], x_d[0, :, t, 6:UC, 0:NH]
                )
            nc.sync.dma_start(wk_sb[0][:], wk_d[0])
            nc.sync.dma_start(wv_sb[0][:], wv_d[0])
            for t in range(2):
                nc.sync.dma_start(x_sb[0][:, t, :, NH:S], x_d[0, :, t, :, NH:S])
            for kc in range(4):
                nc.sync.dma_start(mq_sb[0][:, kc, :], mq_d[0, :, kc, :])
            nc.sync.dma_start(wq_sb[1][:], wq_d[1])
            nc.sync.dma_start(wk_sb[1][:], wk_d[1])
            nc.sync.dma_start(wv_sb[1][:], wv_d[1])
            for kc in range(4, KT):
                nc.sync.dma_start(mq_sb[0][:, kc, :], mq_d[0, :, kc, :])
            for h in range(2, H):
                nc.sync.dma_start(wq_sb[h][:], wq_d[h])
                nc.sync.dma_start(wk_sb[h][:], wk_d[h])
                nc.sync.dma_start(wv_sb[h][:], wv_d[h])
            wo_sb = sb.tile([P, 2, UC, U], F8, tag="wo")
            nc.sync.dma_start(wo_sb[:], wo_d[:])
            if BPC > 1:
                nc.sync.dma_start(x_sb[1][:], x_d[1])
                nc.sync.dma_start(mq_sb[1][:], mq_d[1])

            def emit_mm3(acc, w8, x8, sl):
                """3-term fp8 residual matmul group into PSUM `acc`:
                contraction over all UC blocks, moving cols `sl`."""
                for j in range(0, UC, 2):
                    nc.tensor.matmul(
                        acc[:],
                        w8[:, 1, j : j + 2, :],  # (hi_j, hi_j+1)
                        x8[:, 0, j : j + 2, sl],  # (hi_j, hi_j+1)
                        start=(j == 0),
                        stop=False,
                        perf_mode=DR,
                    )
                for uc in range(UC):
                    nc.tensor.matmul(
                        acc[:],
                        w8[:, :, uc, :],  # (lo, hi)
                        x8[:, :, uc, sl],  # (hi, lo)
                        start=False,
                        stop=(uc == UC - 1),
                        perf_mode=DR,
                    )

            def emit_qkv(b, h):
                qp = sb.tile([P, S], F16, tag="qp", bufs=2, name=f"qp{b}_{h}")
                ks = sb.tile([P, S], F16, tag="ks", bufs=2, name=f"ks{b}_{h}")
                vt = sb.tile([P, KT, C], F16, tag="vt", bufs=2, name=f"vt{b}_{h}")
                for half in range(2):
                    sl = slice(half * NH, (half + 1) * NH)
                    acc = ps.tile([P, NH], F32, tag="qk", bufs=4, name=f"accq{b}_{h}")
                    emit_mm3(acc, wq_sb[h], x_sb[b], sl)
                    nc.scalar.mul(qp[:, sl], acc[:], INV_QK)
                    acc = ps.tile([P, NH], F32, tag="qk", bufs=4, name=f"acck{b}_{h}")
                    emit_mm3(acc, wk_sb[h], x_sb[b], sl)
                    nc.scalar.mul(ks[:, sl], acc[:], INV_QK)
                    # V^T directly: stationary = x block, moving = wv
                    vtp = ps.tile([P, NH], F32, tag="qk", bufs=4, name=f"vtp{b}_{h}")
                    for jj in range(4):
                        kc = half * 4 + jj
                        ksl = slice(kc * P, (kc + 1) * P)
                        csl = slice(jj * C, (jj + 1) * C)
                        for j in range(0, UC, 2):
                            nc.tensor.matmul(
                                vtp[:, csl],
                                x_sb[b][:, 0, j : j + 2, ksl],
                                wv_sb[h][:, 1, j : j + 2, :],
                                start=(j == 0),
                                stop=False,
                                perf_mode=DR,
                            )
                        for uc in range(UC):
                            nc.tensor.matmul(
                                vtp[:, csl],
                                x_sb[b][:, :, uc, ksl],
                                wv_sb[h][:, :, uc, :],
                                start=False,
                                stop=(uc == UC - 1),
                                perf_mode=DR,
                            )
                    nc.scalar.mul(
                        vt[:, half * 4 : (half + 1) * 4, :],
                        vtp[:].rearrange("p (j c) -> p j c", c=C),
                        INV_QK,
                    )
                return qp, ks, vt

            def emit_oproj(ob, occ, ot, half):
                od = out_d[
                    ob, ot * P : (ot + 1) * P, half * NH : (half + 1) * NH
                ]
                sl = slice(half * NH, (half + 1) * NH)
                o_ps = ps.tile(
                    [P, NH], F32, tag="qk", bufs=4, name=f"odf{ob}_{ot}_{half}"
                )
                for j in range(0, UC, 2):
                    nc.tensor.matmul(
                        o_ps[:],
                        wo_sb[:, 1, j : j + 2, ot * P : (ot + 1) * P],
                        occ[:, 0, j : j + 2, sl],
                        start=(j == 0),
                        stop=False,
                        perf_mode=DR,
                    )
                for uc in range(UC):
                    nc.tensor.matmul(
                        o_ps[:],
                        wo_sb[:, :, uc, ot * P : (ot + 1) * P],
                        occ[:, :, uc, sl],
                        start=False,
                        stop=(uc == UC - 1),
                        perf_mode=DR,
                    )
                o_sb = sb.tile(
                    [P, NH], F32, tag="o_sb", bufs=3, name=f"osdf{ob}_{ot}_{half}"
                )
                nc.scalar.mul(o_sb[:], o_ps[:], INV_O)
                nc.sync.dma_start(od, o_sb[:])

            qkv_pre = None
            deferred = []
            for b in range(BPC):
                # cc8 dim1: {0: hi, 1: lo}
                cc = sb.tile([P, 2, UC, S], F8, tag="cc", bufs=2, name=f"cc{b}")
                for h in range(H):
                    if qkv_pre is not None and qkv_pre[0] == (b, h):
                        qp, ks, vt = qkv_pre[1]
                        qkv_pre = None
                    else:
                        qp, ks, vt = emit_qkv(b, h)
                    if b + 1 < BPC and h == H - 1:
                        # pre-emit next batch's head-0 QKV: independent work
                        # the scheduler can use to fill the last head's
                        # attention-tail stalls (AV waiting on the DVE STT
                        # backlog) at the batch boundary
                        qkv_pre = ((b + 1, 0), emit_qkv(b + 1, 0))

                    # logits (transposed) + fused relu*maskq + AV accumulation
                    ch0 = ps.tile([P, NH], F32, tag="ch", bufs=2)
                    ch1 = ps.tile([P, NH], F32, tag="ch", bufs=2)
                    for kc in range(KT):
                        for half, ch in ((0, ch0), (1, ch1)):
                            a_ps = ps.tile([P, NH], F32, tag="a", bufs=2)
                            nc.tensor.matmul(
                                a_ps[:],
                                ks[:, kc * P : (kc + 1) * P],
                                qp[:, half * NH : (half + 1) * NH],
                                start=True,
                                stop=True,
                            )
                            atf = sb.tile([P, NH], F16, tag="atf", bufs=4)
                            nc.vector.scalar_tensor_tensor(
                                atf[:],
                                a_ps[:],
                                0.0,
                                mq_sb[b][:, kc, half * NH : (half + 1) * NH],
                                op0=mybir.AluOpType.max,
                                op1=mybir.AluOpType.mult,
                            )
                            nc.tensor.matmul(
                                ch[:],
                                vt[:, kc, :],
                                atf[:],
                                start=(kc == 0),
                                stop=(kc == KT - 1),
                            )
                    # cc hi = e4m3(ch*SCC) on ACT; lo = residual on DVE
                    for half, ch in ((0, ch0), (1, ch1)):
                        sl = slice(half * NH, (half + 1) * NH)
                        nc.scalar.mul(cc[:, 0, h, sl], ch[:], SCC)
                        nc.vector.scalar_tensor_tensor(
                            cc[:, 1, h, sl],
                            ch[:],
                            SCC,
                            cc[:, 0, h, sl],
                            op0=mybir.AluOpType.mult,
                            op1=mybir.AluOpType.subtract,
                        )

                # ---- output projection (weights already resident) ----
                # the first 2 tiles of every non-final batch are deferred to
                # just before the final batch's out-proj: they are the only
                # independent work available to fill the last head's
                # attention-tail stalls (the DVE STT backlog) there
                defer = (
                    {(0, 0), (0, 1)} if BPC > 1 and b < BPC - 1 else set()
                )
                if b == BPC - 1:
                    for db, dcc, dot, dhalf in deferred:
                        emit_oproj(db, dcc, dot, dhalf)
                for ot in range(UC):
                    for half in range(2):
                        if (ot, half) in defer:
                            deferred.append((b, cc, ot, half))
                            continue
                        od = out_d[
                            b,
                            ot * P : (ot + 1) * P,
                            half * NH : (half + 1) * NH,
                        ]
                        sl = slice(half * NH, (half + 1) * NH)
                        if b == BPC - 1 and ot == UC - 1 and half == 1:
                            # final tile: 4 column-group accumulations in
                            # separate PSUM tiles with interleaved copies, so
                            # after the last matmul only one 128-col copy and
                            # the single DMA remain
                            o_sb = sb.tile([P, NH], F32, tag="o_sb", bufs=3)
                            for j in range(4):
                                jsl = slice(j * P, (j + 1) * P)
                                csl = slice(
                                    half * NH + j * P, half * NH + (j + 1) * P
                                )
                                op_j = ps.tile(
                                    [P, P], F32, tag="qk", bufs=4, name=f"opfin{j}"
                                )
                                for jj in range(0, UC, 2):
                                    nc.tensor.matmul(
                                        op_j[:],
                                        wo_sb[:, 1, jj : jj + 2, ot * P : (ot + 1) * P],
                                        cc[:, 0, jj : jj + 2, csl],
                                        start=(jj == 0),
                                        stop=False,
                                        perf_mode=DR,
                                    )
                                for uc in range(UC):
                                    nc.tensor.matmul(
                                        op_j[:],
                                        wo_sb[:, :, uc, ot * P : (ot + 1) * P],
                                        cc[:, :, uc, csl],
                                        start=False,
                                        stop=(uc == UC - 1),
                                        perf_mode=DR,
                                    )
                                nc.scalar.mul(o_sb[:, jsl], op_j[:], INV_O)
                                if j == 2:
                                    nc.sync.dma_start(
                                        od[:, 0 : 3 * P], o_sb[:, 0 : 3 * P]
                                    )
                            # last chunk alone on SP: its DGE_DMA_DELAY is
                            # 650ns vs ACT's 784, and the transfer is 128 cols
                            nc.sync.dma_start(od[:, 3 * P : NH], o_sb[:, 3 * P : NH])
                        else:
                            o_ps = ps.tile([P, NH], F32, tag="qk", bufs=4)
                            for j in range(0, UC, 2):
                                nc.tensor.matmul(
                                    o_ps[:],
                                    wo_sb[:, 1, j : j + 2, ot * P : (ot + 1) * P],
                                    cc[:, 0, j : j + 2, sl],
                                    start=(j == 0),
                                    stop=False,
                                    perf_mode=DR,
                                )
                            for uc in range(UC):
                                nc.tensor.matmul(
                                    o_ps[:],
                                    wo_sb[:, :, uc, ot * P : (ot + 1) * P],
                                    cc[:, :, uc, sl],
                                    start=False,
                                    stop=(uc == UC - 1),
                                    perf_mode=DR,
                                )
                            o_sb = sb.tile([P, NH], F32, tag="o_sb", bufs=3)
                            nc.scalar.mul(o_sb[:], o_ps[:], INV_O)
                            nc.sync.dma_start(od, o_sb[:])

    nc.compile()
    return nc


_NC_CACHE = None


def _get_nc():
    global _NC_CACHE
    if _NC_CACHE is None:
        _NC_CACHE = build()
    return _NC_CACHE


def _hi_lo(a, scale):
    """e4m3 hi/lo split of a*scale (f32 in, two e4m3 arrays out)."""
    s = (a * np.float32(scale)).astype(np.float32)
    hi = s.astype(E4M3)
    lo = (s - hi.astype(np.float32)).astype(E4M3)
    return hi, lo


def kernel(x, mask, w_qkv, w_out):
    nc = _get_nc()
    x = np.asarray(x, dtype=np.float32)
    mask_b = np.asarray(mask).astype(bool)
    w_qkv = np.asarray(w_qkv, dtype=np.float32)
    w_out = np.asarray(w_out, dtype=np.float32)

    # fp8 scales are compile-time immediates; the asserts guard the e4m3
    # max-normal (240) with >=1.3x margin for these input distributions
    assert np.abs(x).max() * SX <= 240.0
    assert np.abs(w_qkv).max() * SW <= 240.0
    assert np.abs(w_out).max() * SWO <= 240.0

    # maskq[b,k,q] = mask[b,q,k] * scale / max(valid_count[b,q], 1)
    m = mask_b.sum(axis=2).astype(np.float32)  # [B, S]
    qs = SCALE / np.maximum(m, 1.0)
    maskq = mask_b.astype(np.float32) * qs[:, :, None]  # [B, q, k]
    mq = (
        np.ascontiguousarray(
            maskq.transpose(0, 2, 1).reshape(B, KT, P, S).transpose(0, 2, 1, 3)
        ).astype(np.float16)
    )  # [B, P, KT, S]

    xh, xl = _hi_lo(x, SX)  # [B, U, S]
    x8 = np.ascontiguousarray(
        np.stack(
            [xh.reshape(B, UC, P, S), xl.reshape(B, UC, P, S)], axis=1
        ).transpose(0, 3, 1, 2, 4)
    )  # [B, P, 2(hi,lo), UC, S]

    wqkvT = np.ascontiguousarray(w_qkv.T)  # [U, 3U] f32
    packs = []
    for i in range(3):
        w_i = wqkvT[:, i * U : (i + 1) * U]  # [U(in), U(out)]
        hi, lo = _hi_lo(w_i, SW)
        # [2(lo,hi), UC, P, H, C] -> [H, P, 2, UC, C]
        arr = np.stack(
            [lo.reshape(UC, P, H, C), hi.reshape(UC, P, H, C)], axis=0
        ).transpose(3, 2, 0, 1, 4)
        packs.append(np.ascontiguousarray(arr))
    wq, wk, wv = packs
    oh, ol = _hi_lo(w_out.T, SWO)  # [U(in), U(out)]
    wo = np.ascontiguousarray(
        np.stack([ol.reshape(UC, P, U), oh.reshape(UC, P, U)], axis=0).transpose(
            2, 0, 1, 3
        )
    )  # [P, 2(lo,hi), UC, U]

    in_maps = []
    for c in range(NCORES):
        in_maps.append(
            {
                "x8": np.ascontiguousarray(x8[c * BPC : (c + 1) * BPC]),
                "maskq": np.ascontiguousarray(mq[c * BPC : (c + 1) * BPC]),
                "wq": wq,
                "wk": wk,
                "wv": wv,
                "wo": wo,
            }
        )
    res = run_bass_kernel_spmd(nc, in_maps, list(range(NCORES)))
    out = np.concatenate([res.results[c]["out"] for c in range(NCORES)], axis=0)
    return out


# revision 36
# speedup vs baseline: 1.1807x; 1.0221x over previous
"""Multi-head attention (relu + valid-key-count normalization) on 8 TRN2 cores.

Strategy: data-parallel over batch (B=16 -> 2 per core), no collectives.

v3: fp8 DoubleRow 3-term residual matmuls for QKV + output projection.
The TRN2 PE in fp8e4 DoubleRow mode contracts 256 rows per instruction at
0.5 cyc per output column = 4x the f16 MAC rate. A plain fp8 cast is too
lossy (3-6e-2 end-to-end, gate 2e-2), but the 3-term residual scheme
    W ~ Wh + Wl,  X ~ Xh + Xl   (hi = e4m3(t), lo = e4m3(t - hi))
    W^T X ~ Wh^T Xh + (Wl^T Xh + Wh^T Xl)     [drop Wl^T Xl, ~2^-9]
costs 1.5 DoubleRow instructions per 256 contraction rows = 0.75x the f16
cycles, at ~2.2e-3 end-to-end rel err (measured in a bit-exact numpy sim
of this pipeline). Packing needs NO duplication:
  - main terms pair hi-blocks across the contraction dim:
        stat [Wh_j; Wh_j+1], mov [Xh_j; Xh_j+1]
  - both correction terms ride ONE DoubleRow via the cross pairing:
        stat [Wl_j; Wh_j],  mov [Xh_j; Xl_j]
All tensors quantize on a power-of-2 scale chosen so values AND residuals
sit in e4m3 normal range (subnormals at scale 1 destroyed the residuals:
2.8e-2 measured): x*32, w_qkv*1024, w_out*1024, cc*512. Descales fold
into the existing PSUM->SBUF copies (ACT mul) so there is zero extra
elementwise work; cc hi/lo is produced by the existing ACT copy (now a
mul to fp8) plus one DVE STT for the residual.

Logits (contraction = head dim 128 < 256) gains nothing from DoubleRow,
and fp8 attention weights would push DVE/ACT past the PE time, so
logits + AV stay f16: Q/K/V^T emerge from the fp8 QKV PSUM as f16 via
the descaling ACT mul.

Per-core PE floor: QKV 0.75*3 + logits 1 + AV 1 + oproj 0.75 = 5.0 units
of 65536 cyc/batch, 2 batches = 655,360 cyc @2.4GHz = 273.1us (f16 floor
was 327.7us; v2 f16 baseline measured 336.4us). Measured timeline:
284.9us = 7.3us DMA-gated prologue (serial HWDGE 625ns/dma + 360GB/s
transfers for w0/w1 + x-b0 + maskq; bridged by warm-up + filler
transposes so the p-state ramp completes before real work) + 273.2us
gap-free PE stream + 4.4us drain tail. AV in fp8-3term (0.75) was
evaluated and rejected: the atf hi/lo split adds 2 elementwise passes
over the S x S attention tensor per head, pushing DVE+ACT+Pool all to
>91% of the PE time - no realizable gain.

Schedule notes (the Tile static scheduler fills stalls with any READY
later-emitted work, so emission order is priority order):
- next-batch head-0 QKV is emitted right AFTER the last head's
  attention, making it the fill for the attention tail, where AV paces
  at the DVE STT backlog rate (658ns/tile vs 426ns of PE work);
- 6 out-proj tiles of batch 0 are deferred to just before the final
  batch's out-proj as the same kind of fill for batch 1's last head;
- head-1 weights load BEFORE maskq (h1 QKV gates the weave start);
- out stores are f16 (host casts back to f32; +2^-12 rel err, half the
  store DMA bytes), the final out tile drains as 4 column-group PSUMs
  with descales alternating DVE/ACT and a split store so only a 128-col
  chunk remains after the last matmul.
Measured rel err vs the jax reference: ~2.2e-3 (gate 2e-2).
"""
import sys

sys.path.insert(0, "/opt/trn_rl_repo")

import numpy as np
import ml_dtypes

import concourse.bacc as bacc
import concourse.mybir as mybir
import concourse.tile as tile
from concourse.bass_utils import run_bass_kernel_spmd

B, U, S, H, C = 16, 1024, 1024, 8, 128
NCORES = 8
BPC = B // NCORES  # batches per core
SCALE = float(1.0 / np.sqrt(np.float32(C)))
P = 128  # partitions
UC = U // P  # u chunks
QT = S // P  # q tiles
KT = S // P  # k tiles
NH = 512  # matmul free dim (one PSUM bank of f32)
NWARM = 12  # p-state warm-up transposes: bridge PE busy ~1us -> first matmul
NWARM2 = 8  # low-priority filler transposes emitted after head-0 QKV

SX = 32.0  # x fp8 scale (power of 2; absmax*SX must stay <= 240)
SW = 1024.0  # w_qkv fp8 scale
SWO = 1024.0  # w_out fp8 scale
SCC = 512.0  # on-device attention-output (cc) fp8 scale
INV_QK = float(1.0 / (SW * SX))  # QKV PSUM -> f16 descale
INV_O = float(1.0 / (SWO * SCC))  # out-proj PSUM -> f32 descale

F32 = mybir.dt.float32
F16 = mybir.dt.float16
F8 = mybir.dt.float8e4
E4M3 = ml_dtypes.float8_e4m3
DR = mybir.MatmulPerfMode.DoubleRow


def build():
    nc = bacc.Bacc()
    # Host-packed to SBUF layout (partition dim right after batch): every
    # load is one fully-contiguous descriptor per partition.
    # x8 dim2: {0: hi, 1: lo}; weights dim after P: {0: lo, 1: hi}.
    x_d = nc.dram_tensor("x8", [BPC, P, 2, UC, S], F8, kind="ExternalInput")
    mq_d = nc.dram_tensor("maskq", [BPC, P, KT, S], F16, kind="ExternalInput")
    wq_d = nc.dram_tensor("wq", [H, P, 2, UC, C], F8, kind="ExternalInput")
    wk_d = nc.dram_tensor("wk", [H, P, 2, UC, C], F8, kind="ExternalInput")
    wv_d = nc.dram_tensor("wv", [H, P, 2, UC, C], F8, kind="ExternalInput")
    wo_d = nc.dram_tensor("wo", [P, 2, UC, U], F8, kind="ExternalInput")
    out_d = nc.dram_tensor("out", [BPC, U, S], F16, kind="ExternalOutput")

    with tile.TileContext(nc) as tc:
        with (
            tc.tile_pool(name="sb", bufs=1) as sb,
            tc.tile_pool(name="ps", bufs=1, space="PSUM") as ps,
        ):
            # warm-up: a DVE memset (no DMA dependency) feeds a chain of PE
            # transposes that keep the PE continuously busy from ~1us until
            # the first real matmul — otherwise the p-state ramp restarts
            # after the prologue idle and the first ~6us of real matmuls run
            # at 0.65-1.2GHz instead of 2.4GHz.
            wsrc = sb.tile([P, P], F16, tag="wsrc")
            nc.vector.memset(wsrc[:], 0.0)
            warm = ps.tile([P, P], F16, tag="a", bufs=3, name="warm")
            for _ in range(NWARM):
                nc.tensor.transpose(warm[:], wsrc[:], wsrc[:])

            # resident weights; head 0 first so QKV can start ASAP
            wq_sb = [
                sb.tile([P, 2, UC, C], F8, tag=f"wq{h}", name=f"wq_sb{h}")
                for h in range(H)
            ]
            wk_sb = [
                sb.tile([P, 2, UC, C], F8, tag=f"wk{h}", name=f"wk_sb{h}")
                for h in range(H)
            ]
            wv_sb = [
                sb.tile([P, 2, UC, C], F8, tag=f"wv{h}", name=f"wv_sb{h}")
                for h in range(H)
            ]
            x_sb = [
                sb.tile([P, 2, UC, S], F8, tag="x", bufs=2, name=f"x{b}")
                for b in range(BPC)
            ]
            mq_sb = [
                sb.tile([P, KT, S], F16, tag="mq", bufs=2, name=f"mq{b}")
                for b in range(BPC)
            ]
            # ALL loads on the SP queue in strict priority order — the DMA
            # engines device is serialized, so transfer order IS this order.
            # batch-0 x in column halves: the first QKV matmuls need only
            # half 0 of every uc chunk (hi AND lo: the correction DRs
            # interleave with the main DRs in the same accumulation group).
            # every dma_start costs 625ns on the serial HWDGE, so the
            # prologue uses few, large loads ordered by first-consumer:
            # Q mains need wq + x-hi, corr DRs add x-lo, then wk/wv
            nc.sync.dma_start(wq_sb[0][:], wq_d[0])
            nc.sync.dma_start(x_sb[0][:, 0, 0:4, 0:NH], x_d[0, :, 0, 0:4, 0:NH])
            nc.sync.dma_start(wk_sb[0][:], wk_d[0])
            nc.sync.dma_start(x_sb[0][:, 1, 0:4, 0:NH], x_d[0, :, 1, 0:4, 0:NH])
            nc.sync.dma_start(wv_sb[0][:], wv_d[0])
            nc.sync.dma_start(x_sb[0][:, 0, 4:UC, 0:NH], x_d[0, :, 0, 4:UC, 0:NH])
            nc.sync.dma_start(x_sb[0][:, 1, 4:UC, 0:NH], x_d[0, :, 1, 4:UC, 0:NH])
            for t in range(2):
                nc.sync.dma_start(x_sb[0][:, t, :, NH:S], x_d[0, :, t, :, NH:S])
            # head-1 weights BEFORE maskq: h1's QKV is the scheduler's fill
            # for h0's attention (DVE-paced), so it gates the start of the
            # contiguous PE stream; maskq kc-pairs land just-in-time for the
            # h0 STT chain
            nc.sync.dma_start(wq_sb[1][:], wq_d[1])
            nc.sync.dma_start(wk_sb[1][:], wk_d[1])
            nc.sync.dma_start(mq_sb[0][:, 0:2, :], mq_d[0, :, 0:2, :])
            nc.sync.dma_start(wv_sb[1][:], wv_d[1])
            nc.sync.dma_start(mq_sb[0][:, 2:4, :], mq_d[0, :, 2:4, :])
            nc.sync.dma_start(mq_sb[0][:, 4:6, :], mq_d[0, :, 4:6, :])
            nc.sync.dma_start(mq_sb[0][:, 6:8, :], mq_d[0, :, 6:8, :])
            for h in range(2, H):
                nc.sync.dma_start(wq_sb[h][:], wq_d[h])
                nc.sync.dma_start(wk_sb[h][:], wk_d[h])
                nc.sync.dma_start(wv_sb[h][:], wv_d[h])
            wo_sb = sb.tile([P, 2, UC, U], F8, tag="wo")
            nc.sync.dma_start(wo_sb[:], wo_d[:])
            if BPC > 1:
                nc.sync.dma_start(x_sb[1][:], x_d[1])
                nc.sync.dma_start(mq_sb[1][:], mq_d[1])

            def emit_mm3(acc, w8, x8, sl):
                """3-term fp8 residual matmul group into PSUM `acc`:
                contraction over all UC blocks, moving cols `sl`."""
                for j in range(0, UC, 2):
                    nc.tensor.matmul(
                        acc[:],
                        w8[:, 1, j : j + 2, :],  # (hi_j, hi_j+1)
                        x8[:, 0, j : j + 2, sl],  # (hi_j, hi_j+1)
                        start=(j == 0),
                        stop=False,
                        perf_mode=DR,
                    )
                for uc in range(UC):
                    nc.tensor.matmul(
                        acc[:],
                        w8[:, :, uc, :],  # (lo, hi)
                        x8[:, :, uc, sl],  # (hi, lo)
                        start=False,
                        stop=(uc == UC - 1),
                        perf_mode=DR,
                    )

            def emit_qkv(b, h):
                qp = sb.tile([P, S], F16, tag="qp", bufs=2, name=f"qp{b}_{h}")
                ks = sb.tile([P, S], F16, tag="ks", bufs=2, name=f"ks{b}_{h}")
                vt = sb.tile([P, KT, C], F16, tag="vt", bufs=2, name=f"vt{b}_{h}")
                for half in range(2):
                    sl = slice(half * NH, (half + 1) * NH)
                    acc = ps.tile([P, NH], F32, tag="qk", bufs=3, name=f"accq{b}_{h}")
                    emit_mm3(acc, wq_sb[h], x_sb[b], sl)
                    nc.scalar.mul(qp[:, sl], acc[:], INV_QK)
                    acc = ps.tile([P, NH], F32, tag="qk", bufs=3, name=f"acck{b}_{h}")
                    emit_mm3(acc, wk_sb[h], x_sb[b], sl)
                    nc.scalar.mul(ks[:, sl], acc[:], INV_QK)
                    # V^T directly: stationary = x block, moving = wv
                    vtp = ps.tile([P, NH], F32, tag="qk", bufs=3, name=f"vtp{b}_{h}")
                    for jj in range(4):
                        kc = half * 4 + jj
                        ksl = slice(kc * P, (kc + 1) * P)
                        csl = slice(jj * C, (jj + 1) * C)
                        for j in range(0, UC, 2):
                            nc.tensor.matmul(
                                vtp[:, csl],
                                x_sb[b][:, 0, j : j + 2, ksl],
                                wv_sb[h][:, 1, j : j + 2, :],
                                start=(j == 0),
                                stop=False,
                                perf_mode=DR,
                            )
                        for uc in range(UC):
                            nc.tensor.matmul(
                                vtp[:, csl],
                                x_sb[b][:, :, uc, ksl],
                                wv_sb[h][:, :, uc, :],
                                start=False,
                                stop=(uc == UC - 1),
                                perf_mode=DR,
                            )
                    nc.scalar.mul(
                        vt[:, half * 4 : (half + 1) * 4, :],
                        vtp[:].rearrange("p (j c) -> p j c", c=C),
                        INV_QK,
                    )
                return qp, ks, vt

            def emit_oproj(ob, occ, ot, half):
                od = out_d[
                    ob, ot * P : (ot + 1) * P, half * NH : (half + 1) * NH
                ]
                sl = slice(half * NH, (half + 1) * NH)
                o_ps = ps.tile(
                    [P, NH], F32, tag="qk", bufs=3, name=f"odf{ob}_{ot}_{half}"
                )
                for j in range(0, UC, 2):
                    nc.tensor.matmul(
                        o_ps[:],
                        wo_sb[:, 1, j : j + 2, ot * P : (ot + 1) * P],
                        occ[:, 0, j : j + 2, sl],
                        start=(j == 0),
                        stop=False,
                        perf_mode=DR,
                    )
                for uc in range(UC):
                    nc.tensor.matmul(
                        o_ps[:],
                        wo_sb[:, :, uc, ot * P : (ot + 1) * P],
                        occ[:, :, uc, sl],
                        start=False,
                        stop=(uc == UC - 1),
                        perf_mode=DR,
                    )
                o_sb = sb.tile(
                    [P, NH], F16, tag="o_sb", bufs=4, name=f"osdf{ob}_{ot}_{half}"
                )
                nc.scalar.mul(o_sb[:], o_ps[:], INV_O)
                nc.sync.dma_start(od, o_sb[:])

            # head-0 QKV emitted first, then more warm-up transposes: the
            # static scheduler slots these into the prologue's DMA-wait
            # windows (they are always-ready, lower-priority filler), which
            # keeps the PE busy so the p-state ramp reaches 2.4GHz
            qkv_pre = ((0, 0), emit_qkv(0, 0))
            for _ in range(NWARM2):
                nc.tensor.transpose(warm[:], wsrc[:], wsrc[:])
            deferred = []
            for b in range(BPC):
                # cc8 dim1: {0: hi, 1: lo}
                cc = sb.tile([P, 2, UC, S], F8, tag="cc", bufs=2, name=f"cc{b}")
                for h in range(H):
                    if qkv_pre is not None and qkv_pre[0] == (b, h):
                        qp, ks, vt = qkv_pre[1]
                        qkv_pre = None
                    else:
                        qp, ks, vt = emit_qkv(b, h)
                    # logits (transposed) + fused relu*maskq + AV accumulation
                    ch0 = ps.tile([P, NH], F32, tag="ch", bufs=2)
                    ch1 = ps.tile([P, NH], F32, tag="ch", bufs=2)
                    for kc in range(KT):
                        for half, ch in ((0, ch0), (1, ch1)):
                            a_ps = ps.tile([P, NH], F32, tag="a", bufs=3)
                            nc.tensor.matmul(
                                a_ps[:],
                                ks[:, kc * P : (kc + 1) * P],
                                qp[:, half * NH : (half + 1) * NH],
                                start=True,
                                stop=True,
                            )
                            atf = sb.tile([P, NH], F16, tag="atf", bufs=4)
                            nc.vector.scalar_tensor_tensor(
                                atf[:],
                                a_ps[:],
                                0.0,
                                mq_sb[b][:, kc, half * NH : (half + 1) * NH],
                                op0=mybir.AluOpType.max,
                                op1=mybir.AluOpType.mult,
                            )
                            nc.tensor.matmul(
                                ch[:],
                                vt[:, kc, :],
                                atf[:],
                                start=(kc == 0),
                                stop=(kc == KT - 1),
                            )
                    if b + 1 < BPC and h == H - 1:
                        # emit next batch's head-0 QKV right AFTER the last
                        # head's attention: in scheduler priority order this
                        # is the ready work that fills the attention-tail
                        # stalls (AV pacing at the DVE STT backlog rate)
                        qkv_pre = ((b + 1, 0), emit_qkv(b + 1, 0))

                    # cc hi = e4m3(ch*SCC) on ACT; lo = residual on DVE
                    for half, ch in ((0, ch0), (1, ch1)):
                        sl = slice(half * NH, (half + 1) * NH)
                        nc.scalar.mul(cc[:, 0, h, sl], ch[:], SCC)
                        nc.vector.scalar_tensor_tensor(
                            cc[:, 1, h, sl],
                            ch[:],
                            SCC,
                            cc[:, 0, h, sl],
                            op0=mybir.AluOpType.mult,
                            op1=mybir.AluOpType.subtract,
                        )

                # ---- output projection (weights already resident) ----
                # the first 2 tiles of every non-final batch are deferred to
                # just before the final batch's out-proj: they are the only
                # independent work available to fill the last head's
                # attention-tail stalls (the DVE STT backlog) there
                defer = (
                    {(0, 0), (0, 1), (1, 0), (1, 1), (2, 0), (2, 1)}
                    if BPC > 1 and b < BPC - 1
                    else set()
                )
                if b == BPC - 1:
                    for db, dcc, dot, dhalf in deferred:
                        emit_oproj(db, dcc, dot, dhalf)
                for ot in range(UC):
                    for half in range(2):
                        if (ot, half) in defer:
                            deferred.append((b, cc, ot, half))
                            continue
                        od = out_d[
                            b,
                            ot * P : (ot + 1) * P,
                            half * NH : (half + 1) * NH,
                        ]
                        sl = slice(half * NH, (half + 1) * NH)
                        if b == BPC - 1 and ot == UC - 1 and half == 1:
                            # final tile: 4 column-group accumulations in
                            # separate PSUM tiles with interleaved copies, so
                            # after the last matmul only one 128-col copy and
                            # the single DMA remain
                            o_sb = sb.tile([P, NH], F16, tag="o_sb", bufs=4)
                            for j in range(4):
                                jsl = slice(j * P, (j + 1) * P)
                                csl = slice(
                                    half * NH + j * P, half * NH + (j + 1) * P
                                )
                                op_j = ps.tile(
                                    [P, P], F32, tag="qk", bufs=3, name=f"opfin{j}"
                                )
                                for jj in range(0, UC, 2):
                                    nc.tensor.matmul(
                                        op_j[:],
                                        wo_sb[:, 1, jj : jj + 2, ot * P : (ot + 1) * P],
                                        cc[:, 0, jj : jj + 2, csl],
                                        start=(jj == 0),
                                        stop=False,
                                        perf_mode=DR,
                                    )
                                for uc in range(UC):
                                    nc.tensor.matmul(
                                        op_j[:],
                                        wo_sb[:, :, uc, ot * P : (ot + 1) * P],
                                        cc[:, :, uc, csl],
                                        start=False,
                                        stop=(uc == UC - 1),
                                        perf_mode=DR,
                                    )
                                # alternate the drain descales DVE/ACT so
                                # they pipeline instead of queueing behind
                                # one engine's in-order backlog
                                if j % 2 == 0:
                                    nc.vector.tensor_scalar_mul(
                                        o_sb[:, jsl], op_j[:], INV_O
                                    )
                                else:
                                    nc.scalar.mul(o_sb[:, jsl], op_j[:], INV_O)
                                if j == 1:
                                    nc.sync.dma_start(
                                        od[:, 0 : 2 * P], o_sb[:, 0 : 2 * P]
                                    )
                            # last chunk alone on SP: its DGE_DMA_DELAY is
                            # 650ns vs ACT's 784, and the transfer is small
                            nc.sync.dma_start(od[:, 2 * P : NH], o_sb[:, 2 * P : NH])
                        else:
                            o_ps = ps.tile([P, NH], F32, tag="qk", bufs=3)
                            for j in range(0, UC, 2):
                                nc.tensor.matmul(
                                    o_ps[:],
                                    wo_sb[:, 1, j : j + 2, ot * P : (ot + 1) * P],
                                    cc[:, 0, j : j + 2, sl],
                                    start=(j == 0),
                                    stop=False,
                                    perf_mode=DR,
                                )
                            for uc in range(UC):
                                nc.tensor.matmul(
                                    o_ps[:],
                                    wo_sb[:, :, uc, ot * P : (ot + 1) * P],
                                    cc[:, :, uc, sl],
                                    start=False,
                                    stop=(uc == UC - 1),
                                    perf_mode=DR,
                                )
                            o_sb = sb.tile([P, NH], F16, tag="o_sb", bufs=4)
                            nc.scalar.mul(o_sb[:], o_ps[:], INV_O)
                            nc.sync.dma_start(od, o_sb[:])

    nc.compile()
    return nc


_NC_CACHE = None


def _get_nc():
    global _NC_CACHE
    if _NC_CACHE is None:
        _NC_CACHE = build()
    return _NC_CACHE


def _hi_lo(a, scale):
    """e4m3 hi/lo split of a*scale (f32 in, two e4m3 arrays out)."""
    s = (a * np.float32(scale)).astype(np.float32)
    hi = s.astype(E4M3)
    lo = (s - hi.astype(np.float32)).astype(E4M3)
    return hi, lo


def kernel(x, mask, w_qkv, w_out):
    nc = _get_nc()
    x = np.asarray(x, dtype=np.float32)
    mask_b = np.asarray(mask).astype(bool)
    w_qkv = np.asarray(w_qkv, dtype=np.float32)
    w_out = np.asarray(w_out, dtype=np.float32)

    # fp8 scales are compile-time immediates; the asserts guard the e4m3
    # max-normal (240) with >=1.3x margin for these input distributions
    assert np.abs(x).max() * SX <= 240.0
    assert np.abs(w_qkv).max() * SW <= 240.0
    assert np.abs(w_out).max() * SWO <= 240.0

    # maskq[b,k,q] = mask[b,q,k] * scale / max(valid_count[b,q], 1)
    m = mask_b.sum(axis=2).astype(np.float32)  # [B, S]
    qs = SCALE / np.maximum(m, 1.0)
    maskq = mask_b.astype(np.float32) * qs[:, :, None]  # [B, q, k]
    mq = (
        np.ascontiguousarray(
            maskq.transpose(0, 2, 1).reshape(B, KT, P, S).transpose(0, 2, 1, 3)
        ).astype(np.float16)
    )  # [B, P, KT, S]

    xh, xl = _hi_lo(x, SX)  # [B, U, S]
    x8 = np.ascontiguousarray(
        np.stack(
            [xh.reshape(B, UC, P, S), xl.reshape(B, UC, P, S)], axis=1
        ).transpose(0, 3, 1, 2, 4)
    )  # [B, P, 2(hi,lo), UC, S]

    wqkvT = np.ascontiguousarray(w_qkv.T)  # [U, 3U] f32
    packs = []
    for i in range(3):
        w_i = wqkvT[:, i * U : (i + 1) * U]  # [U(in), U(out)]
        hi, lo = _hi_lo(w_i, SW)
        # [2(lo,hi), UC, P, H, C] -> [H, P, 2, UC, C]
        arr = np.stack(
            [lo.reshape(UC, P, H, C), hi.reshape(UC, P, H, C)], axis=0
        ).transpose(3, 2, 0, 1, 4)
        packs.append(np.ascontiguousarray(arr))
    wq, wk, wv = packs
    oh, ol = _hi_lo(w_out.T, SWO)  # [U(in), U(out)]
    wo = np.ascontiguousarray(
        np.stack([ol.reshape(UC, P, U), oh.reshape(UC, P, U)], axis=0).transpose(
            2, 0, 1, 3
        )
    )  # [P, 2(lo,hi), UC, U]

    in_maps = []
    for c in range(NCORES):
        in_maps.append(
            {
                "x8": np.ascontiguousarray(x8[c * BPC : (c + 1) * BPC]),
                "maskq": np.ascontiguousarray(mq[c * BPC : (c + 1) * BPC]),
                "wq": wq,
                "wk": wk,
                "wv": wv,
                "wo": wo,
            }
        )
    res = run_bass_kernel_spmd(nc, in_maps, list(range(NCORES)))
    out = np.concatenate([res.results[c]["out"] for c in range(NCORES)], axis=0).astype(np.float32)
    return out
